# revision 33
# baseline (speedup 1.0000x reference)
"""AttentionBlock (GroupNorm -> QKV -> single-head attention -> proj -> residual)
as a Bass/Tile kernel for 8 Trainium2 NeuronCores.

Sharding: 8 cores = 4 batches x 2 query-halves. Each core receives its batch's
x[b] as [C, N] with columns rotated so that its query half occupies columns
0:N/2 (group-norm statistics and attention are invariant to a permutation of
the key/value positions, so every core runs the identical SPMD program).
Each core computes group-norm + full K/V + its half of the queries, runs
flash-style attention entirely on-chip, and writes y_half = (x + proj)[:, :N/2].
"""

import os
import sys

import numpy as np

for _p in ("/opt/trn_rl_repo", "/root/.axon_site/_ro/trn_rl_repo"):
    if os.path.isdir(_p) and _p not in sys.path:
        sys.path.insert(0, _p)

import concourse.bacc as bacc
import concourse.mybir as mybir
import concourse.tile as tile
from concourse import bass_utils

# Problem shape (hardcoded per harness contract).
B, C, H, W = 4, 256, 64, 64
N = H * W            # 4096 key/value positions
NQ = N // 2          # 2048 queries per core
G = 32               # group-norm groups
GSIZE = (C // G) * N # elements per group
EPS = 1e-5
SCALE = float(C) ** -0.5  # single head -> d = C
P = 128
CCH = C // P         # 2 channel chunks of 128
N_CORES = 8

FB = 512             # matmul moving-operand free-dim (one PSUM bank of f32)
N_IC = NQ // FB      # 4 query chunks per core
N_JC = N // P        # 32 key chunks of 128
N_KB = N // FB       # 8 key blocks of 512 (for the k matmul)

F32 = mybir.dt.float32
# Storage dtype of the big matmul operands (q/k/vT/P/weights).
# float32r streams 1 col/cycle on TensorE (vs 4 for float32) at N>=256;
# producers must write f32r-rounded outputs, so the tiles are declared f32r.
DT = mybir.dt.float32r
MM_R = False
SKEW = 2

_CACHE = {}


def _build():
    if "nc" in _CACHE:
        return _CACHE["nc"]

    nc = bacc.Bacc(
        "TRN2",
        target_bir_lowering=False,
        debug=False,
        enable_asserts=False,
        num_devices=N_CORES,
    )

    xb = nc.dram_tensor("xb", [C, N], F32, kind="ExternalInput").ap()
    wq = nc.dram_tensor("wq", [C, C], F32, kind="ExternalInput").ap()   # Wq^T
    wk = nc.dram_tensor("wk", [C, C], F32, kind="ExternalInput").ap()   # Wk^T
    wv = nc.dram_tensor("wv", [C, C], F32, kind="ExternalInput").ap()   # Wv^T
    wp = nc.dram_tensor("wp", [C, C], F32, kind="ExternalInput").ap()   # Wproj^T
    qb = nc.dram_tensor("qb", [C], F32, kind="ExternalInput").ap()
    kb = nc.dram_tensor("kb", [C], F32, kind="ExternalInput").ap()
    vb = nc.dram_tensor("vb", [C], F32, kind="ExternalInput").ap()
    pb = nc.dram_tensor("pb", [C], F32, kind="ExternalInput").ap()
    nw = nc.dram_tensor("nw", [C], F32, kind="ExternalInput").ap()
    nb = nc.dram_tensor("nb", [C], F32, kind="ExternalInput").ap()
    mask = nc.dram_tensor("mask", [P, G // CCH], F32, kind="ExternalInput").ap()
    maskT = nc.dram_tensor("maskT", [G // CCH, P], F32, kind="ExternalInput").ap()
    y = nc.dram_tensor("y", [C, NQ], F32, kind="ExternalOutput").ap()

    with tile.TileContext(nc) as tc:
        _emit(nc, tc, xb, wq, wk, wv, wp, qb, kb, vb, pb, nw, nb, mask, maskT, y)

    nc.compile()
    _CACHE["nc"] = nc
    return nc


def _emit(nc, tc, xb, wq, wk, wv, wp, qb, kb, vb, pb, nw, nb, mask, maskT, y):
    from contextlib import ExitStack

    GG = G // CCH  # 16 groups per channel-chunk
    R = (lambda ap: ap.bitcast(mybir.dt.float32r)) if MM_R else (lambda ap: ap)

    with ExitStack() as ctx:
        big = ctx.enter_context(tc.tile_pool(name="big", bufs=1))
        singles = ctx.enter_context(tc.tile_pool(name="singles", bufs=1))

        # ---- load x (in column blocks, pipelined with stats) ----
        NBLK = 8
        BLK = N // NBLK
        xr = xb.rearrange("(cc p) n -> p cc n", p=P)
        x_sb = big.tile([P, CCH, N], F32)
        for blk in range(NBLK):
            eng = (nc.sync, nc.scalar)[blk % 2]
            eng.dma_start(
                out=x_sb[:, :, blk * BLK:(blk + 1) * BLK],
                in_=xr[:, :, blk * BLK:(blk + 1) * BLK])

        wq_sb = singles.tile([P, CCH, C], DT)
        nc.sync.dma_start(
            out=wq_sb,
            in_=wq.rearrange("(cc p) o -> p cc o", p=P).bitcast(DT))
        wk_sb = singles.tile([P, CCH, C], DT)
        nc.sync.dma_start(
            out=wk_sb,
            in_=wk.rearrange("(cc p) o -> p cc o", p=P).bitcast(DT))
        wv_sb = singles.tile([P, CCH, C], DT)
        nc.sync.dma_start(
            out=wv_sb,
            in_=wv.rearrange("(cc p) o -> p cc o", p=P).bitcast(DT))
        wp_sb = singles.tile([P, CCH, C], DT)
        nc.sync.dma_start(
            out=wp_sb,
            in_=wp.rearrange("(cc p) o -> p cc o", p=P).bitcast(DT))

        qb_sb = singles.tile([P, CCH], F32)
        nc.sync.dma_start(out=qb_sb, in_=qb.rearrange("(cc p) -> p cc", p=P))
        kb_sb = singles.tile([P, CCH], F32)
        nc.sync.dma_start(out=kb_sb, in_=kb.rearrange("(cc p) -> p cc", p=P))
        pb_sb = singles.tile([P, CCH], F32)
        nc.sync.dma_start(out=pb_sb, in_=pb.rearrange("(cc p) -> p cc", p=P))
        nw_sb = singles.tile([P, CCH], F32)
        nc.sync.dma_start(out=nw_sb, in_=nw.rearrange("(cc p) -> p cc", p=P))
        nb_sb = singles.tile([P, CCH], F32)
        nc.sync.dma_start(out=nb_sb, in_=nb.rearrange("(cc p) -> p cc", p=P))
        vb_bc = singles.tile([P, C], F32)
        import concourse.bass as bass_mod
        vb_b = bass_mod.AP(tensor=vb.tensor, offset=vb.offset,
                           ap=[[0, P]] + list(vb.ap))
        nc.sync.dma_start(out=vb_bc, in_=vb_b)

        mask_sb = singles.tile([P, GG], F32)
        nc.sync.dma_start(out=mask_sb, in_=mask)
        maskT_sb = singles.tile([GG, P], F32)
        nc.sync.dma_start(out=maskT_sb, in_=maskT)

        ones_f32 = singles.tile([P, 1], F32)
        nc.vector.memset(ones_f32, 1.0)
        ones_sb = singles.tile([P, 1], DT)
        nc.vector.tensor_copy(out=ones_sb, in_=ones_f32)
        eps_sb = singles.tile([GG, 1], F32)
        nc.vector.memset(eps_sb, EPS)

        # ---- group norm ----
        xn_sb = big.tile([P, CCH, N], DT)

        with (
            tc.tile_pool(name="gn", bufs=2) as gn,
            tc.tile_pool(name="ps_gn", bufs=2, space="PSUM") as ps_gn,
        ):
            rs = gn.tile([P, CCH, NBLK, 2], F32)  # per-row per-block [sum, sumsq]
            for blk in range(NBLK):
                for ch in range(CCH):
                    xs = x_sb[:, ch, blk * BLK:(blk + 1) * BLK]
                    nc.vector.tensor_reduce(
                        out=rs[:, ch, blk, 0:1], in_=xs,
                        axis=mybir.AxisListType.X, op=mybir.AluOpType.add,
                    )
                    sq2 = gn.tile([P, BLK], F32, tag="sq2")
                    nc.scalar.activation(
                        out=sq2, in_=xs,
                        func=mybir.ActivationFunctionType.Square,
                        accum_out=rs[:, ch, blk, 1:2],
                    )
            # cross-partition group sums: [16g, (ch, blk, stat)]
            ps_st = ps_gn.tile([GG, CCH, NBLK, 2], F32)
            nc.tensor.matmul(ps_st, mask_sb, rs, start=True, stop=True)
            stc = gn.tile([GG, CCH, 2], F32)
            nc.vector.tensor_reduce(
                out=stc, in_=ps_st.rearrange("g c b s -> g c s b"),
                axis=mybir.AxisListType.X, op=mybir.AluOpType.add,
            )

            st = gn.tile([GG, CCH, 2], F32)   # [mean, E[x^2]]
            nc.scalar.mul(out=st, in_=stc, mul=1.0 / GSIZE)
            msq = gn.tile([GG, CCH], F32)
            nc.vector.tensor_mul(out=msq, in0=st[:, :, 0], in1=st[:, :, 0])
            var = gn.tile([GG, CCH], F32)
            nc.vector.tensor_sub(out=var, in0=st[:, :, 1], in1=msq)
            sd = gn.tile([GG, CCH], F32)
            nc.scalar.activation(
                out=sd, in_=var, func=mybir.ActivationFunctionType.Sqrt,
                bias=eps_sb, scale=1.0,
            )
            rstd = gn.tile([GG, CCH], F32)
            nc.vector.reciprocal(out=rstd, in_=sd)

            pk = gn.tile([GG, CCH, 2], F32)   # [mean, rstd]
            nc.vector.tensor_copy(out=pk[:, :, 0], in_=st[:, :, 0])
            nc.vector.tensor_copy(out=pk[:, :, 1], in_=rstd)
            ps_bc = ps_gn.tile([P, CCH, 2], F32)
            nc.tensor.matmul(ps_bc, maskT_sb, pk, start=True, stop=True)

            scl = gn.tile([P, CCH], F32)      # rstd * norm_w  per channel
            nc.vector.tensor_mul(out=scl, in0=ps_bc[:, :, 1], in1=nw_sb)
            tmp = gn.tile([P, CCH], F32)
            nc.vector.tensor_mul(out=tmp, in0=ps_bc[:, :, 0], in1=scl)
            shf = gn.tile([P, CCH], F32)      # norm_b - mean*rstd*norm_w
            nc.vector.tensor_sub(out=shf, in0=nb_sb, in1=tmp)

            for blk in range(NBLK):
                for ch in range(CCH):
                    eng = nc.vector if (blk * CCH + ch) % 2 == 0 else nc.gpsimd
                    eng.tensor_scalar(
                        out=xn_sb[:, ch, blk * BLK:(blk + 1) * BLK],
                        in0=x_sb[:, ch, blk * BLK:(blk + 1) * BLK],
                        scalar1=scl[:, ch:ch + 1], scalar2=shf[:, ch:ch + 1],
                        op0=mybir.AluOpType.mult, op1=mybir.AluOpType.add,
                    )


        # ---- qkv ----
        q_sb = big.tile([P, CCH, NQ], DT)
        k_sb = big.tile([P, CCH, N], DT)
        vT_sb = big.tile([P, N_JC, C], DT)

        with tc.tile_pool(name="ps_mm", bufs=3, space="PSUM") as ps_mm:
            for blk in range(NBLK):
                fcs = range(blk * (N // NBLK // FB), (blk + 1) * (N // NBLK // FB))
                for oc in range(CCH):
                    for ic in fcs:
                        if ic >= N_IC:
                            continue
                        ps = ps_mm.tile([P, FB], F32)
                        for cc in range(CCH):
                            nc.tensor.matmul(
                                ps, R(wq_sb[:, cc, oc * P:(oc + 1) * P]),
                                R(xn_sb[:, cc, ic * FB:(ic + 1) * FB]),
                                start=(cc == 0), stop=(cc == CCH - 1),
                            )
                        nc.vector.tensor_scalar_add(
                            out=q_sb[:, oc, ic * FB:(ic + 1) * FB], in0=ps,
                            scalar1=qb_sb[:, oc:oc + 1],
                        )
                    for kc in fcs:
                        ps = ps_mm.tile([P, FB], F32)
                        for cc in range(CCH):
                            nc.tensor.matmul(
                                ps, R(wk_sb[:, cc, oc * P:(oc + 1) * P]),
                                R(xn_sb[:, cc, kc * FB:(kc + 1) * FB]),
                                start=(cc == 0), stop=(cc == CCH - 1),
                            )
                        nc.vector.tensor_scalar_add(
                            out=k_sb[:, oc, kc * FB:(kc + 1) * FB], in0=ps,
                            scalar1=kb_sb[:, oc:oc + 1],
                        )
                for jc in range(blk * (N_JC // NBLK), (blk + 1) * (N_JC // NBLK)):
                    ps = ps_mm.tile([P, C], F32)
                    for cc in range(CCH):
                        nc.tensor.matmul(
                            ps, R(xn_sb[:, cc, jc * P:(jc + 1) * P]), R(wv_sb[:, cc, :]),
                            start=(cc == 0), stop=(cc == CCH - 1),
                        )
                    nc.vector.tensor_add(out=vT_sb[:, jc, :], in0=ps, in1=vb_bc)


        # ---- attention + proj + residual ----
        with (
            tc.tile_pool(name="pt", bufs=8) as pp,
            tc.tile_pool(name="att", bufs=2) as att,
            tc.tile_pool(name="outp", bufs=3) as outp,
            tc.tile_pool(name="ps_s", bufs=3, space="PSUM") as ps_s,
            tc.tile_pool(name="ps_o", bufs=1, space="PSUM") as ps_o,
            tc.tile_pool(name="ps_l", bufs=1, space="PSUM") as ps_l,
            tc.tile_pool(name="ps_p", bufs=2, space="PSUM") as ps_p,
        ):
            yr = y.rearrange("(oc p) i -> p oc i", p=P)
            for ic in range(N_IC):
                psO = att.tile([P, CCH, FB], DT, tag="psO_sb")
                ps_o0 = ps_o.tile([P, FB], F32, tag="o0")
                ps_o1 = ps_o.tile([P, FB], F32, tag="o1")
                lacc_d = att.tile([P, FB], DT, tag="lacc_d")
                lacc_g = att.tile([P, FB], DT, tag="lacc_g")

                def emit_ol(jc, pt):
                    first, last = jc == 0, jc == N_JC - 1
                    nc.tensor.matmul(ps_o0, R(vT_sb[:, jc, 0:P]), R(pt),
                                     start=first, stop=last)
                    nc.tensor.matmul(ps_o1, R(vT_sb[:, jc, P:C]), R(pt),
                                     start=first, stop=last)
                    eng, acc = ((nc.vector, lacc_d) if jc % 2 == 0
                                else (nc.gpsimd, lacc_g))
                    if jc < 2:
                        eng.tensor_copy(out=acc, in_=pt)
                    else:
                        eng.tensor_add(out=acc, in0=acc, in1=pt)

                pend = []
                for jc in range(N_JC):
                    pss = ps_s.tile([P, FB], F32)
                    for dc in range(CCH):
                        nc.tensor.matmul(
                            pss, R(k_sb[:, dc, jc * P:(jc + 1) * P]),
                            R(q_sb[:, dc, ic * FB:(ic + 1) * FB]),
                            start=(dc == 0), stop=(dc == CCH - 1),
                        )
                    pt = pp.tile([P, FB], DT)
                    nc.scalar.activation(
                        out=pt, in_=pss,
                        func=mybir.ActivationFunctionType.Exp, scale=SCALE,
                    )
                    pend.append((jc, pt))
                    if len(pend) > SKEW:
                        emit_ol(*pend.pop(0))
                for e in pend:
                    emit_ol(*e)
                nc.vector.tensor_add(out=lacc_d, in0=lacc_d, in1=lacc_g)
                psl = ps_l.tile([1, FB], F32)
                nc.tensor.matmul(psl, R(ones_sb), R(lacc_d), start=True, stop=True)

                # 1/l broadcast to all partitions
                rcp = att.tile([1, FB], F32, tag="rcp")
                nc.vector.reciprocal(out=rcp, in_=psl)
                rbc = att.tile([P, FB], F32, tag="rbc")
                nc.gpsimd.partition_broadcast(rbc, rcp)

                # drain O to SBUF, then proj
                nc.vector.tensor_copy(out=psO[:, 0, :], in_=ps_o0)
                nc.vector.tensor_copy(out=psO[:, 1, :], in_=ps_o1)
                for oc in range(CCH):
                    psp = ps_p.tile([P, FB], F32)
                    for dc in range(CCH):
                        nc.tensor.matmul(
                            psp, R(wp_sb[:, dc, oc * P:(oc + 1) * P]), R(psO[:, dc, :]),
                            start=(dc == 0), stop=(dc == CCH - 1),
                        )
                    t = outp.tile([P, FB], F32)
                    nc.vector.tensor_mul(out=t, in0=psp, in1=rbc)
                    nc.vector.tensor_scalar_add(out=t, in0=t,
                                                scalar1=pb_sb[:, oc:oc + 1])
                    nc.vector.tensor_add(
                        out=t, in0=t, in1=x_sb[:, oc, ic * FB:(ic + 1) * FB])
                    nc.sync.dma_start(out=yr[:, oc, ic * FB:(ic + 1) * FB], in_=t)


def _host_inputs(x, norm_w, norm_b, qkv_w, qkv_b, proj_w, proj_b):
    f = np.float32
    wq = np.ascontiguousarray(qkv_w[0:C].T, dtype=f)
    wk = np.ascontiguousarray(qkv_w[C:2 * C].T, dtype=f)
    wv = np.ascontiguousarray(qkv_w[2 * C:3 * C].T, dtype=f)
    wp = np.ascontiguousarray(proj_w.T, dtype=f)
    qb, kb, vb = (np.ascontiguousarray(qkv_b[i * C:(i + 1) * C], dtype=f)
                  for i in range(3))
    GG = G // CCH
    mask = np.zeros((P, GG), dtype=f)
    mask[np.arange(P), np.arange(P) // (C // G)] = 1.0
    maskT = np.ascontiguousarray(mask.T)

    shared = dict(
        wq=wq, wk=wk, wv=wv, wp=wp, qb=qb, kb=kb, vb=vb,
        pb=np.ascontiguousarray(proj_b, dtype=f),
        nw=np.ascontiguousarray(norm_w, dtype=f),
        nb=np.ascontiguousarray(norm_b, dtype=f),
        mask=mask, maskT=maskT,
    )

    in_maps = []
    for core in range(N_CORES):
        b, h = core // 2, core % 2
        xv = np.asarray(x[b], dtype=f).reshape(C, N)
        xrot = np.ascontiguousarray(np.roll(xv, -h * NQ, axis=1))
        in_maps.append(dict(shared, xb=xrot))
    return in_maps


def kernel(x, norm_w, norm_b, qkv_w, qkv_b, proj_w, proj_b, num_heads=1):
    x, norm_w, norm_b, qkv_w, qkv_b, proj_w, proj_b = (
        np.asarray(a) for a in (x, norm_w, norm_b, qkv_w, qkv_b, proj_w, proj_b))
    nc = _build()
    in_maps = _host_inputs(x, norm_w, norm_b, qkv_w, qkv_b, proj_w, proj_b)
    res = bass_utils.run_bass_kernel_spmd(nc, in_maps, core_ids=list(range(N_CORES)))
    out = np.empty((B, C, N), dtype=np.float32)
    for core in range(N_CORES):
        b, h = core // 2, core % 2
        out[b, :, h * NQ:(h + 1) * NQ] = res.results[core]["y"]
    return out.reshape(B, C, H, W)


# revision 41
# speedup vs baseline: 1.0775x; 1.0775x over previous
"""AttentionBlock (GroupNorm -> QKV -> single-head attention -> proj -> residual)
as a Bass/Tile kernel for 8 Trainium2 NeuronCores.

Sharding: 8 cores = 4 batches x 2 query-halves. Each core receives its batch's
x[b] as [C, N] with columns rotated so that its query half occupies columns
0:N/2 (group-norm statistics and attention are invariant to a permutation of
the key/value positions, so every core runs the identical SPMD program).
Each core computes group-norm + full K/V + its half of the queries, runs
flash-style attention entirely on-chip, and writes y_half = (x + proj)[:, :N/2].
"""

import os
import sys

import numpy as np

for _p in ("/opt/trn_rl_repo", "/root/.axon_site/_ro/trn_rl_repo"):
    if os.path.isdir(_p) and _p not in sys.path:
        sys.path.insert(0, _p)

import concourse.bacc as bacc
import concourse.mybir as mybir
import concourse.tile as tile
from concourse import bass_utils

# Problem shape (hardcoded per harness contract).
B, C, H, W = 4, 256, 64, 64
N = H * W            # 4096 key/value positions
NQ = N // 2          # 2048 queries per core
G = 32               # group-norm groups
GSIZE = (C // G) * N # elements per group
EPS = 1e-5
SCALE = float(C) ** -0.5  # single head -> d = C
P = 128
CCH = C // P         # 2 channel chunks of 128
N_CORES = 8

FB = 512             # matmul moving-operand free-dim (one PSUM bank of f32)
N_IC = NQ // FB      # 4 query chunks per core
N_JC = N // P        # 32 key chunks of 128
N_KB = N // FB       # 8 key blocks of 512 (for the k matmul)

F32 = mybir.dt.float32
# Storage dtype of the big matmul operands (q/k/vT/P/weights).
# float32r streams 1 col/cycle on TensorE (vs 4 for float32) at N>=256;
# producers must write f32r-rounded outputs, so the tiles are declared f32r.
DT = mybir.dt.float32r
MM_R = False
SKEW = 2

_CACHE = {}


def _build():
    if "nc" in _CACHE:
        return _CACHE["nc"]

    nc = bacc.Bacc(
        "TRN2",
        target_bir_lowering=False,
        debug=False,
        enable_asserts=False,
        num_devices=N_CORES,
    )

    xb = nc.dram_tensor("xb", [C, N], F32, kind="ExternalInput").ap()
    wq = nc.dram_tensor("wq", [C, C], F32, kind="ExternalInput").ap()   # Wq^T
    wk = nc.dram_tensor("wk", [C, C], F32, kind="ExternalInput").ap()   # Wk^T
    wv = nc.dram_tensor("wv", [C, C], F32, kind="ExternalInput").ap()   # Wv^T
    wp = nc.dram_tensor("wp", [C, C], F32, kind="ExternalInput").ap()   # Wproj^T
    qb = nc.dram_tensor("qb", [C], F32, kind="ExternalInput").ap()
    kb = nc.dram_tensor("kb", [C], F32, kind="ExternalInput").ap()
    vb = nc.dram_tensor("vb", [C], F32, kind="ExternalInput").ap()
    pb = nc.dram_tensor("pb", [C], F32, kind="ExternalInput").ap()
    nw = nc.dram_tensor("nw", [C], F32, kind="ExternalInput").ap()
    nb = nc.dram_tensor("nb", [C], F32, kind="ExternalInput").ap()
    mask = nc.dram_tensor("mask", [P, G // CCH], F32, kind="ExternalInput").ap()
    maskT = nc.dram_tensor("maskT", [G // CCH, P], F32, kind="ExternalInput").ap()
    y = nc.dram_tensor("y", [C, NQ], F32, kind="ExternalOutput").ap()

    with tile.TileContext(nc) as tc:
        _emit(nc, tc, xb, wq, wk, wv, wp, qb, kb, vb, pb, nw, nb, mask, maskT, y)

    nc.compile()
    _CACHE["nc"] = nc
    return nc


def _emit(nc, tc, xb, wq, wk, wv, wp, qb, kb, vb, pb, nw, nb, mask, maskT, y):
    from contextlib import ExitStack

    GG = G // CCH  # 16 groups per channel-chunk
    R = (lambda ap: ap.bitcast(mybir.dt.float32r)) if MM_R else (lambda ap: ap)

    with ExitStack() as ctx:
        big = ctx.enter_context(tc.tile_pool(name="big", bufs=1))
        singles = ctx.enter_context(tc.tile_pool(name="singles", bufs=1))

        # ---- loads: small constants first (they gate the stats chain),
        # then x in column blocks (pipelined with stats), then big weights ----
        NBLK = 8
        BLK = N // NBLK

        warm = singles.tile([1, 1], F32)
        nc.vector.memset(warm, 1.0)
        warm2 = singles.tile([1, 1], F32)
        nc.scalar.activation(out=warm2, in_=warm,
                             func=mybir.ActivationFunctionType.Sqrt)

        qb_sb = singles.tile([P, CCH], F32)
        nc.gpsimd.dma_start(out=qb_sb, in_=qb.rearrange("(cc p) -> p cc", p=P))
        kb_sb = singles.tile([P, CCH], F32)
        nc.gpsimd.dma_start(out=kb_sb, in_=kb.rearrange("(cc p) -> p cc", p=P))
        pb_sb = singles.tile([P, CCH], F32)
        nc.gpsimd.dma_start(out=pb_sb, in_=pb.rearrange("(cc p) -> p cc", p=P))
        nw_sb = singles.tile([P, CCH], F32)
        nc.gpsimd.dma_start(out=nw_sb, in_=nw.rearrange("(cc p) -> p cc", p=P))
        nb_sb = singles.tile([P, CCH], F32)
        nc.gpsimd.dma_start(out=nb_sb, in_=nb.rearrange("(cc p) -> p cc", p=P))
        vb_bc = singles.tile([P, C], F32)
        import concourse.bass as bass_mod
        vb_b = bass_mod.AP(tensor=vb.tensor, offset=vb.offset,
                           ap=[[0, P]] + list(vb.ap))
        nc.gpsimd.dma_start(out=vb_bc, in_=vb_b)
        mask_sb = singles.tile([P, GG], F32)
        nc.gpsimd.dma_start(out=mask_sb, in_=mask)
        maskT_sb = singles.tile([GG, P], F32)
        nc.gpsimd.dma_start(out=maskT_sb, in_=maskT)

        xr = xb.rearrange("(cc p) n -> p cc n", p=P)
        x_sb = big.tile([P, CCH, N], F32)
        for blk in range(NBLK):
            eng = (nc.sync, nc.scalar)[blk % 2]
            eng.dma_start(
                out=x_sb[:, :, blk * BLK:(blk + 1) * BLK],
                in_=xr[:, :, blk * BLK:(blk + 1) * BLK])

        wq_sb = singles.tile([P, CCH, C], DT)
        nc.sync.dma_start(
            out=wq_sb,
            in_=wq.rearrange("(cc p) o -> p cc o", p=P).bitcast(DT))
        wk_sb = singles.tile([P, CCH, C], DT)
        nc.sync.dma_start(
            out=wk_sb,
            in_=wk.rearrange("(cc p) o -> p cc o", p=P).bitcast(DT))
        wv_sb = singles.tile([P, CCH, C], DT)
        nc.sync.dma_start(
            out=wv_sb,
            in_=wv.rearrange("(cc p) o -> p cc o", p=P).bitcast(DT))
        wp_sb = singles.tile([P, CCH, C], DT)
        nc.sync.dma_start(
            out=wp_sb,
            in_=wp.rearrange("(cc p) o -> p cc o", p=P).bitcast(DT))

        ones_f32 = singles.tile([P, 1], F32)
        nc.vector.memset(ones_f32, 1.0)
        ones_sb = singles.tile([P, 1], DT)
        nc.vector.tensor_copy(out=ones_sb, in_=ones_f32)
        eps_sb = singles.tile([GG, 1], F32)
        nc.vector.memset(eps_sb, EPS)

        # ---- group norm ----
        xn_sb = big.tile([P, CCH, N], DT)

        with (
            tc.tile_pool(name="gn", bufs=2) as gn,
            tc.tile_pool(name="ps_gn", bufs=2, space="PSUM") as ps_gn,
        ):
            rs = gn.tile([P, CCH, NBLK, 2], F32)  # per-row per-block [sum, sumsq]
            for blk in range(NBLK):
                for ch in range(CCH):
                    xs = x_sb[:, ch, blk * BLK:(blk + 1) * BLK]
                    nc.vector.tensor_reduce(
                        out=rs[:, ch, blk, 0:1], in_=xs,
                        axis=mybir.AxisListType.X, op=mybir.AluOpType.add,
                    )
                    sq2 = gn.tile([P, BLK], F32, tag="sq2")
                    nc.scalar.activation(
                        out=sq2, in_=xs,
                        func=mybir.ActivationFunctionType.Square,
                        accum_out=rs[:, ch, blk, 1:2],
                    )
            # cross-partition group sums: [16g, (ch, blk, stat)]
            ps_st = ps_gn.tile([GG, CCH, NBLK, 2], F32)
            nc.tensor.matmul(ps_st, mask_sb, rs, start=True, stop=True)
            stc = gn.tile([GG, CCH, 2], F32)
            nc.vector.tensor_reduce(
                out=stc, in_=ps_st.rearrange("g c b s -> g c s b"),
                axis=mybir.AxisListType.X, op=mybir.AluOpType.add,
            )

            st = gn.tile([GG, CCH, 2], F32)   # [mean, E[x^2]]
            nc.scalar.mul(out=st, in_=stc, mul=1.0 / GSIZE)
            msq = gn.tile([GG, CCH], F32)
            nc.vector.tensor_mul(out=msq, in0=st[:, :, 0], in1=st[:, :, 0])
            var = gn.tile([GG, CCH], F32)
            nc.vector.tensor_sub(out=var, in0=st[:, :, 1], in1=msq)
            sd = gn.tile([GG, CCH], F32)
            nc.scalar.activation(
                out=sd, in_=var, func=mybir.ActivationFunctionType.Sqrt,
                bias=eps_sb, scale=1.0,
            )
            rstd = gn.tile([GG, CCH], F32)
            nc.vector.reciprocal(out=rstd, in_=sd)

            pk = gn.tile([GG, CCH, 2], F32)   # [mean, rstd]
            nc.vector.tensor_copy(out=pk[:, :, 0], in_=st[:, :, 0])
            nc.vector.tensor_copy(out=pk[:, :, 1], in_=rstd)
            ps_bc = ps_gn.tile([P, CCH, 2], F32)
            nc.tensor.matmul(ps_bc, maskT_sb, pk, start=True, stop=True)

            scl = gn.tile([P, CCH], F32)      # rstd * norm_w  per channel
            nc.vector.tensor_mul(out=scl, in0=ps_bc[:, :, 1], in1=nw_sb)
            tmp = gn.tile([P, CCH], F32)
            nc.vector.tensor_mul(out=tmp, in0=ps_bc[:, :, 0], in1=scl)
            shf = gn.tile([P, CCH], F32)      # norm_b - mean*rstd*norm_w
            nc.vector.tensor_sub(out=shf, in0=nb_sb, in1=tmp)

            for blk in range(NBLK):
                for ch in range(CCH):
                    eng = nc.vector if (blk * CCH + ch) % 2 == 0 else nc.gpsimd
                    eng.tensor_scalar(
                        out=xn_sb[:, ch, blk * BLK:(blk + 1) * BLK],
                        in0=x_sb[:, ch, blk * BLK:(blk + 1) * BLK],
                        scalar1=scl[:, ch:ch + 1], scalar2=shf[:, ch:ch + 1],
                        op0=mybir.AluOpType.mult, op1=mybir.AluOpType.add,
                    )


        # ---- qkv (fused with attention for query-chunk 0) ----
        q_sb = big.tile([P, CCH, NQ], DT)
        k_sb = big.tile([P, CCH, N], DT)
        vT_sb = big.tile([P, N_JC, C], DT)

        yr = y.rearrange("(oc p) i -> p oc i", p=P)
        with (
            tc.tile_pool(name="pt", bufs=8) as pp,
            tc.tile_pool(name="att", bufs=2) as att,
            tc.tile_pool(name="outp", bufs=3) as outp,
            tc.tile_pool(name="ps_s", bufs=3, space="PSUM") as ps_s,
            tc.tile_pool(name="ps_o", bufs=1, space="PSUM") as ps_o,
            tc.tile_pool(name="ps_l", bufs=1, space="PSUM") as ps_l,
        ):
            st8 = {}

            def att_begin(ic):
                st8["ic"] = ic
                st8["psO"] = att.tile([P, CCH, FB], DT, tag="psO_sb", name="psO")
                st8["o0"] = ps_o.tile([P, FB], F32, tag="o0", name="pso0")
                st8["o1"] = ps_o.tile([P, FB], F32, tag="o1", name="pso1")
                st8["ld"] = att.tile([P, FB], DT, tag="lacc_d", name="lacc_d")
                st8["lg"] = att.tile([P, FB], DT, tag="lacc_g", name="lacc_g")
                st8["pend"] = []

            def emit_ol(jc, pt):
                first, last = jc == 0, jc == N_JC - 1
                nc.tensor.matmul(st8["o0"], R(vT_sb[:, jc, 0:P]), R(pt),
                                 start=first, stop=last)
                nc.tensor.matmul(st8["o1"], R(vT_sb[:, jc, P:C]), R(pt),
                                 start=first, stop=last)
                eng, acc = ((nc.vector, st8["ld"]) if jc % 2 == 0
                            else (nc.gpsimd, st8["lg"]))
                if jc < 2:
                    eng.tensor_copy(out=acc, in_=pt)
                else:
                    eng.tensor_add(out=acc, in0=acc, in1=pt)

            def att_jcs(jcs):
                ic = st8["ic"]
                for jc in jcs:
                    pss = ps_s.tile([P, FB], F32)
                    for dc in range(CCH):
                        nc.tensor.matmul(
                            pss, R(k_sb[:, dc, jc * P:(jc + 1) * P]),
                            R(q_sb[:, dc, ic * FB:(ic + 1) * FB]),
                            start=(dc == 0), stop=(dc == CCH - 1),
                        )
                    pt = pp.tile([P, FB], DT)
                    nc.scalar.activation(
                        out=pt, in_=pss,
                        func=mybir.ActivationFunctionType.Exp, scale=SCALE,
                    )
                    st8["pend"].append((jc, pt))
                    if len(st8["pend"]) > SKEW:
                        emit_ol(*st8["pend"].pop(0))

            def att_end(ps_p):
                ic = st8["ic"]
                for e in st8["pend"]:
                    emit_ol(*e)
                nc.vector.tensor_add(out=st8["ld"], in0=st8["ld"], in1=st8["lg"])
                psl = ps_l.tile([1, FB], F32)
                nc.tensor.matmul(psl, R(ones_sb), R(st8["ld"]),
                                 start=True, stop=True)
                rcp = att.tile([1, FB], F32, tag="rcp")
                nc.vector.reciprocal(out=rcp, in_=psl)
                rbc = att.tile([P, FB], F32, tag="rbc")
                nc.gpsimd.partition_broadcast(rbc, rcp)

                psO = st8["psO"]
                nc.vector.tensor_copy(out=psO[:, 0, :], in_=st8["o0"])
                nc.vector.tensor_copy(out=psO[:, 1, :], in_=st8["o1"])
                for oc in range(CCH):
                    psp = ps_p.tile([P, FB], F32)
                    for dc in range(CCH):
                        nc.tensor.matmul(
                            psp, R(wp_sb[:, dc, oc * P:(oc + 1) * P]),
                            R(psO[:, dc, :]),
                            start=(dc == 0), stop=(dc == CCH - 1),
                        )
                    t = outp.tile([P, FB], F32)
                    nc.vector.tensor_mul(out=t, in0=psp, in1=rbc)
                    nc.vector.tensor_scalar_add(out=t, in0=t,
                                                scalar1=pb_sb[:, oc:oc + 1])
                    nc.vector.tensor_add(
                        out=t, in0=t, in1=x_sb[:, oc, ic * FB:(ic + 1) * FB])
                    nc.sync.dma_start(out=yr[:, oc, ic * FB:(ic + 1) * FB],
                                      in_=t)

            with tc.tile_pool(name="ps_mm", bufs=2, space="PSUM") as ps_mm:
                att_begin(0)
                for blk in range(NBLK):
                    fcs = range(blk * (N // NBLK // FB),
                                (blk + 1) * (N // NBLK // FB))
                    for oc in range(CCH):
                        for icq in fcs:
                            if icq >= N_IC:
                                continue
                            ps = ps_mm.tile([P, FB], F32)
                            for cc in range(CCH):
                                nc.tensor.matmul(
                                    ps, R(wq_sb[:, cc, oc * P:(oc + 1) * P]),
                                    R(xn_sb[:, cc, icq * FB:(icq + 1) * FB]),
                                    start=(cc == 0), stop=(cc == CCH - 1),
                                )
                            nc.vector.tensor_scalar_add(
                                out=q_sb[:, oc, icq * FB:(icq + 1) * FB], in0=ps,
                                scalar1=qb_sb[:, oc:oc + 1],
                            )
                        for kc in fcs:
                            ps = ps_mm.tile([P, FB], F32)
                            for cc in range(CCH):
                                nc.tensor.matmul(
                                    ps, R(wk_sb[:, cc, oc * P:(oc + 1) * P]),
                                    R(xn_sb[:, cc, kc * FB:(kc + 1) * FB]),
                                    start=(cc == 0), stop=(cc == CCH - 1),
                                )
                            nc.vector.tensor_scalar_add(
                                out=k_sb[:, oc, kc * FB:(kc + 1) * FB], in0=ps,
                                scalar1=kb_sb[:, oc:oc + 1],
                            )
                    for jc in range(blk * (N_JC // NBLK),
                                    (blk + 1) * (N_JC // NBLK)):
                        ps = ps_mm.tile([P, C], F32)
                        for cc in range(CCH):
                            nc.tensor.matmul(
                                ps, R(xn_sb[:, cc, jc * P:(jc + 1) * P]),
                                R(wv_sb[:, cc, :]),
                                start=(cc == 0), stop=(cc == CCH - 1),
                            )
                        nc.vector.tensor_add(out=vT_sb[:, jc, :], in0=ps,
                                             in1=vb_bc)
                    # interleave query-chunk 0 attention for this block's keys
                    att_jcs(range(blk * (N_JC // NBLK),
                                  (blk + 1) * (N_JC // NBLK)))

            with tc.tile_pool(name="ps_p", bufs=2, space="PSUM") as ps_p:
                att_end(ps_p)
                for ic in range(1, N_IC):
                    att_begin(ic)
                    att_jcs(range(N_JC))
                    att_end(ps_p)


def _host_inputs(x, norm_w, norm_b, qkv_w, qkv_b, proj_w, proj_b):
    f = np.float32
    wq = np.ascontiguousarray(qkv_w[0:C].T, dtype=f)
    wk = np.ascontiguousarray(qkv_w[C:2 * C].T, dtype=f)
    wv = np.ascontiguousarray(qkv_w[2 * C:3 * C].T, dtype=f)
    wp = np.ascontiguousarray(proj_w.T, dtype=f)
    qb, kb, vb = (np.ascontiguousarray(qkv_b[i * C:(i + 1) * C], dtype=f)
                  for i in range(3))
    GG = G // CCH
    mask = np.zeros((P, GG), dtype=f)
    mask[np.arange(P), np.arange(P) // (C // G)] = 1.0
    maskT = np.ascontiguousarray(mask.T)

    shared = dict(
        wq=wq, wk=wk, wv=wv, wp=wp, qb=qb, kb=kb, vb=vb,
        pb=np.ascontiguousarray(proj_b, dtype=f),
        nw=np.ascontiguousarray(norm_w, dtype=f),
        nb=np.ascontiguousarray(norm_b, dtype=f),
        mask=mask, maskT=maskT,
    )

    in_maps = []
    for core in range(N_CORES):
        b, h = core // 2, core % 2
        xv = np.asarray(x[b], dtype=f).reshape(C, N)
        xrot = np.ascontiguousarray(np.roll(xv, -h * NQ, axis=1))
        in_maps.append(dict(shared, xb=xrot))
    return in_maps


def kernel(x, norm_w, norm_b, qkv_w, qkv_b, proj_w, proj_b, num_heads=1):
    x, norm_w, norm_b, qkv_w, qkv_b, proj_w, proj_b = (
        np.asarray(a) for a in (x, norm_w, norm_b, qkv_w, qkv_b, proj_w, proj_b))
    nc = _build()
    in_maps = _host_inputs(x, norm_w, norm_b, qkv_w, qkv_b, proj_w, proj_b)
    res = bass_utils.run_bass_kernel_spmd(nc, in_maps, core_ids=list(range(N_CORES)))
    out = np.empty((B, C, N), dtype=np.float32)
    for core in range(N_CORES):
        b, h = core // 2, core % 2
        out[b, :, h * NQ:(h + 1) * NQ] = res.results[core]["y"]
    return out.reshape(B, C, H, W)


# revision 58
# speedup vs baseline: 1.0913x; 1.0128x over previous
"""AttentionBlock (GroupNorm -> QKV -> single-head attention -> proj -> residual)
as a Bass/Tile kernel for 8 Trainium2 NeuronCores.

Sharding: 8 cores = 4 batches x 2 query-halves. Each core receives its batch's
x[b] as [C, N] with columns rotated so that its query half occupies columns
0:N/2 (group-norm statistics and attention are invariant to a permutation of
the key/value positions, so every core runs the identical SPMD program).
Each core computes group-norm + full K/V + its half of the queries, runs
flash-style attention entirely on-chip, and writes y_half = (x + proj)[:, :N/2].
"""

import os
import sys

import numpy as np

for _p in ("/opt/trn_rl_repo", "/root/.axon_site/_ro/trn_rl_repo"):
    if os.path.isdir(_p) and _p not in sys.path:
        sys.path.insert(0, _p)

import concourse.bacc as bacc
import concourse.mybir as mybir
import concourse.tile as tile
from concourse import bass_utils

# Problem shape (hardcoded per harness contract).
B, C, H, W = 4, 256, 64, 64
N = H * W            # 4096 key/value positions
NQ = N // 2          # 2048 queries per core
G = 32               # group-norm groups
GSIZE = (C // G) * N # elements per group
EPS = 1e-5
SCALE = float(C) ** -0.5  # single head -> d = C
P = 128
CCH = C // P         # 2 channel chunks of 128
N_CORES = 8

FB = 512             # matmul moving-operand free-dim (one PSUM bank of f32)
N_IC = NQ // FB      # 4 query chunks per core
N_JC = N // P        # 32 key chunks of 128
N_KB = N // FB       # 8 key blocks of 512 (for the k matmul)

F32 = mybir.dt.float32
# Storage dtype of the big matmul operands (q/k/vT/P/weights).
# float32r streams 1 col/cycle on TensorE (vs 4 for float32) at N>=256;
# producers must write f32r-rounded outputs, so the tiles are declared f32r.
DT = mybir.dt.float32r
MM_R = False
SKEW = 2

_CACHE = {}


def _build():
    if "nc" in _CACHE:
        return _CACHE["nc"]

    nc = bacc.Bacc(
        "TRN2",
        target_bir_lowering=False,
        debug=False,
        enable_asserts=False,
        num_devices=N_CORES,
    )

    xb = nc.dram_tensor("xb", [C, N], F32, kind="ExternalInput").ap()
    wq = nc.dram_tensor("wq", [C, C], F32, kind="ExternalInput").ap()   # Wq^T
    wk = nc.dram_tensor("wk", [C, C], F32, kind="ExternalInput").ap()   # Wk^T
    wv = nc.dram_tensor("wv", [C, C], F32, kind="ExternalInput").ap()   # Wv^T
    wp = nc.dram_tensor("wp", [C, C], F32, kind="ExternalInput").ap()   # Wproj^T
    qb = nc.dram_tensor("qb", [C], F32, kind="ExternalInput").ap()
    kb = nc.dram_tensor("kb", [C], F32, kind="ExternalInput").ap()
    vb = nc.dram_tensor("vb", [C], F32, kind="ExternalInput").ap()
    pb = nc.dram_tensor("pb", [C], F32, kind="ExternalInput").ap()
    nw = nc.dram_tensor("nw", [C], F32, kind="ExternalInput").ap()
    nb = nc.dram_tensor("nb", [C], F32, kind="ExternalInput").ap()
    mask = nc.dram_tensor("mask", [P, G // CCH], F32, kind="ExternalInput").ap()
    maskT = nc.dram_tensor("maskT", [G // CCH, P], F32, kind="ExternalInput").ap()
    y = nc.dram_tensor("y", [C, NQ], F32, kind="ExternalOutput").ap()

    with tile.TileContext(nc) as tc:
        _emit(nc, tc, xb, wq, wk, wv, wp, qb, kb, vb, pb, nw, nb, mask, maskT, y)

    nc.compile()
    _CACHE["nc"] = nc
    return nc


def _emit(nc, tc, xb, wq, wk, wv, wp, qb, kb, vb, pb, nw, nb, mask, maskT, y):
    from contextlib import ExitStack

    GG = G // CCH  # 16 groups per channel-chunk
    R = (lambda ap: ap.bitcast(mybir.dt.float32r)) if MM_R else (lambda ap: ap)

    with ExitStack() as ctx:
        big = ctx.enter_context(tc.tile_pool(name="big", bufs=1))
        singles = ctx.enter_context(tc.tile_pool(name="singles", bufs=1))

        # ---- loads: small constants first (they gate the stats chain),
        # then x in column blocks (pipelined with stats), then big weights ----
        NBLK = 8
        BLK = N // NBLK

        warm = singles.tile([1, 1], F32)
        nc.vector.memset(warm, 1.0)
        warm2 = singles.tile([1, 1], F32)
        nc.scalar.activation(out=warm2, in_=warm,
                             func=mybir.ActivationFunctionType.Sqrt)

        qb_sb = singles.tile([P, CCH], F32)
        nc.gpsimd.dma_start(out=qb_sb, in_=qb.rearrange("(cc p) -> p cc", p=P))
        kb_sb = singles.tile([P, CCH], F32)
        nc.gpsimd.dma_start(out=kb_sb, in_=kb.rearrange("(cc p) -> p cc", p=P))
        pb_sb = singles.tile([P, CCH], F32)
        nc.gpsimd.dma_start(out=pb_sb, in_=pb.rearrange("(cc p) -> p cc", p=P))
        nw_sb = singles.tile([P, CCH], F32)
        nc.gpsimd.dma_start(out=nw_sb, in_=nw.rearrange("(cc p) -> p cc", p=P))
        nb_sb = singles.tile([P, CCH], F32)
        nc.gpsimd.dma_start(out=nb_sb, in_=nb.rearrange("(cc p) -> p cc", p=P))
        vb_bc = singles.tile([P, C], F32)
        import concourse.bass as bass_mod
        vb_b = bass_mod.AP(tensor=vb.tensor, offset=vb.offset,
                           ap=[[0, P]] + list(vb.ap))
        nc.gpsimd.dma_start(out=vb_bc, in_=vb_b)
        mask_sb = singles.tile([P, GG], F32)
        nc.gpsimd.dma_start(out=mask_sb, in_=mask)
        maskT_sb = singles.tile([GG, P], F32)
        nc.gpsimd.dma_start(out=maskT_sb, in_=maskT)

        xr = xb.rearrange("(cc p) n -> p cc n", p=P)
        x_sb = big.tile([P, CCH, N], F32)
        for blk in range(NBLK):
            nc.sync.dma_start(
                out=x_sb[:, :, blk * BLK:(blk + 1) * BLK],
                in_=xr[:, :, blk * BLK:(blk + 1) * BLK])

        wq_sb = singles.tile([P, CCH, C], DT)
        nc.sync.dma_start(
            out=wq_sb,
            in_=wq.rearrange("(cc p) o -> p cc o", p=P).bitcast(DT))
        wk_sb = singles.tile([P, CCH, C], DT)
        nc.sync.dma_start(
            out=wk_sb,
            in_=wk.rearrange("(cc p) o -> p cc o", p=P).bitcast(DT))
        wv_sb = singles.tile([P, CCH, C], DT)
        nc.sync.dma_start(
            out=wv_sb,
            in_=wv.rearrange("(cc p) o -> p cc o", p=P).bitcast(DT))
        wp_sb = singles.tile([P, CCH, C], DT)
        nc.sync.dma_start(
            out=wp_sb,
            in_=wp.rearrange("(cc p) o -> p cc o", p=P).bitcast(DT))

        ones_f32 = singles.tile([P, 1], F32)
        nc.vector.memset(ones_f32, 1.0)
        ones_sb = singles.tile([P, 1], DT)
        nc.vector.tensor_copy(out=ones_sb, in_=ones_f32)
        eps_sb = singles.tile([GG, 1], F32)
        nc.vector.memset(eps_sb, EPS)

        # ---- group norm ----
        xn_sb = big.tile([P, CCH, N], DT)

        with (
            tc.tile_pool(name="gn", bufs=2) as gn,
            tc.tile_pool(name="ps_gn", bufs=2, space="PSUM") as ps_gn,
        ):
            # stats units: 1024-col pairs early (fewer serial ACT ops while
            # DMA-paced), 512-col singles for the last two blocks (short tail)
            units = [(0, 2), (2, 2), (4, 2), (6, 1), (7, 1)]
            NPAIR = len(units)
            rs = gn.tile([P, CCH, NPAIR, 2], F32)  # per-row per-unit [sum, sumsq]
            for pr, (b0, nb_) in enumerate(units):
                for ch in range(CCH):
                    xs = x_sb[:, ch, b0 * BLK:(b0 + nb_) * BLK]
                    nc.vector.tensor_reduce(
                        out=rs[:, ch, pr, 0:1], in_=xs,
                        axis=mybir.AxisListType.X, op=mybir.AluOpType.add,
                    )
                    sq2 = gn.tile([P, 2 * BLK], F32, tag="sq2")
                    nc.scalar.activation(
                        out=sq2[:, :nb_ * BLK], in_=xs,
                        func=mybir.ActivationFunctionType.Square,
                        accum_out=rs[:, ch, pr, 1:2],
                    )
            # cross-partition group sums: [16g, (ch, blk, stat)]
            ps_st = ps_gn.tile([GG, CCH, NPAIR, 2], F32)
            nc.tensor.matmul(ps_st, mask_sb, rs, start=True, stop=True)
            stc = gn.tile([GG, CCH, 2], F32)
            nc.vector.tensor_reduce(
                out=stc, in_=ps_st.rearrange("g c b s -> g c s b"),
                axis=mybir.AxisListType.X, op=mybir.AluOpType.add,
            )

            st = stc                          # [mean, E[x^2]] (mask pre-scaled)
            msq = gn.tile([GG, CCH], F32)
            nc.vector.tensor_mul(out=msq, in0=st[:, :, 0], in1=st[:, :, 0])
            var = gn.tile([GG, CCH], F32)
            nc.vector.tensor_sub(out=var, in0=st[:, :, 1], in1=msq)
            sd = gn.tile([GG, CCH], F32)
            nc.scalar.activation(
                out=sd, in_=var, func=mybir.ActivationFunctionType.Sqrt,
                bias=eps_sb, scale=1.0,
            )
            rstd = gn.tile([GG, CCH], F32)
            nc.vector.reciprocal(out=rstd, in_=sd)

            pk = gn.tile([GG, CCH, 2], F32)   # [mean, rstd]
            nc.vector.tensor_copy(out=pk[:, :, 0], in_=st[:, :, 0])
            nc.vector.tensor_copy(out=pk[:, :, 1], in_=rstd)
            ps_bc = ps_gn.tile([P, CCH, 2], F32)
            nc.tensor.matmul(ps_bc, maskT_sb, pk, start=True, stop=True)

            scl = gn.tile([P, CCH], F32)      # rstd * norm_w  per channel
            nc.vector.tensor_mul(out=scl, in0=ps_bc[:, :, 1], in1=nw_sb)
            tmp = gn.tile([P, CCH], F32)
            nc.vector.tensor_mul(out=tmp, in0=ps_bc[:, :, 0], in1=scl)
            shf = gn.tile([P, CCH], F32)      # norm_b - mean*rstd*norm_w
            nc.vector.tensor_sub(out=shf, in0=nb_sb, in1=tmp)

            for blk in range(NBLK):
                for ch in range(CCH):
                    eng = nc.vector if (blk * CCH + ch) % 2 == 0 else nc.gpsimd
                    eng.tensor_scalar(
                        out=xn_sb[:, ch, blk * BLK:(blk + 1) * BLK],
                        in0=x_sb[:, ch, blk * BLK:(blk + 1) * BLK],
                        scalar1=scl[:, ch:ch + 1], scalar2=shf[:, ch:ch + 1],
                        op0=mybir.AluOpType.mult, op1=mybir.AluOpType.add,
                    )


        # residual carries proj_b: fold pb into x's query half once (Pool,
        # off the per-query-chunk epilogue chain)
        for oc in range(CCH):
            nc.gpsimd.tensor_scalar_add(
                out=x_sb[:, oc, 0:NQ], in0=x_sb[:, oc, 0:NQ],
                scalar1=pb_sb[:, oc:oc + 1])

        # ---- qkv (fused with attention for query-chunk 0) ----
        q_sb = big.tile([P, CCH, NQ], DT)
        k_sb = big.tile([P, CCH, N], DT)
        vT_sb = big.tile([P, N_JC, C], DT)

        yr = y.rearrange("(oc p) i -> p oc i", p=P)
        with (
            tc.tile_pool(name="pt", bufs=8) as pp,
            tc.tile_pool(name="att", bufs=2) as att,
            tc.tile_pool(name="outp", bufs=3) as outp,
            tc.tile_pool(name="ps_s", bufs=3, space="PSUM") as ps_s,
            tc.tile_pool(name="ps_o", bufs=1, space="PSUM") as ps_o,
            tc.tile_pool(name="ps_l", bufs=1, space="PSUM") as ps_l,
        ):
            st8 = {}

            def att_begin(ic):
                st8["ic"] = ic
                st8["psO"] = att.tile([P, CCH, FB], DT, tag="psO_sb", name="psO")
                st8["o0"] = ps_o.tile([P, FB], F32, tag="o0", name="pso0")
                st8["o1"] = ps_o.tile([P, FB], F32, tag="o1", name="pso1")
                st8["ld"] = att.tile([P, FB], DT, tag="lacc_d", name="lacc_d")
                st8["lg"] = att.tile([P, FB], DT, tag="lacc_g", name="lacc_g")
                st8["pend"] = []

            def emit_ol(jc, pt):
                first, last = jc == 0, jc == N_JC - 1
                nc.tensor.matmul(st8["o0"], R(vT_sb[:, jc, 0:P]), R(pt),
                                 start=first, stop=last)
                nc.tensor.matmul(st8["o1"], R(vT_sb[:, jc, P:C]), R(pt),
                                 start=first, stop=last)
                eng, acc = ((nc.vector, st8["ld"]) if jc % 2 == 0
                            else (nc.gpsimd, st8["lg"]))
                if jc < 2:
                    eng.tensor_copy(out=acc, in_=pt)
                else:
                    eng.tensor_add(out=acc, in0=acc, in1=pt)

            def att_jcs(jcs):
                ic = st8["ic"]
                for jc in jcs:
                    pss = ps_s.tile([P, FB], F32)
                    for dc in range(CCH):
                        nc.tensor.matmul(
                            pss, R(k_sb[:, dc, jc * P:(jc + 1) * P]),
                            R(q_sb[:, dc, ic * FB:(ic + 1) * FB]),
                            start=(dc == 0), stop=(dc == CCH - 1),
                        )
                    pt = pp.tile([P, FB], DT)
                    nc.scalar.activation(
                        out=pt, in_=pss,
                        func=mybir.ActivationFunctionType.Exp, scale=SCALE,
                    )
                    st8["pend"].append((jc, pt))
                    if len(st8["pend"]) > SKEW:
                        emit_ol(*st8["pend"].pop(0))

            def att_end(ps_p):
                ic = st8["ic"]
                for e in st8["pend"]:
                    emit_ol(*e)
                nc.vector.tensor_add(out=st8["ld"], in0=st8["ld"], in1=st8["lg"])
                psl = ps_l.tile([1, FB], F32)
                nc.tensor.matmul(psl, R(ones_sb), R(st8["ld"]),
                                 start=True, stop=True)
                rcp = att.tile([1, FB], F32, tag="rcp")
                nc.vector.reciprocal(out=rcp, in_=psl)
                rbc = att.tile([P, FB], F32, tag="rbc")
                nc.gpsimd.partition_broadcast(rbc, rcp)

                psO = st8["psO"]
                nc.vector.tensor_copy(out=psO[:, 0, :], in_=st8["o0"])
                nc.vector.tensor_copy(out=psO[:, 1, :], in_=st8["o1"])
                for oc in range(CCH):
                    psp = ps_p.tile([P, FB], F32)
                    for dc in range(CCH):
                        nc.tensor.matmul(
                            psp, R(wp_sb[:, dc, oc * P:(oc + 1) * P]),
                            R(psO[:, dc, :]),
                            start=(dc == 0), stop=(dc == CCH - 1),
                        )
                    t = outp.tile([P, FB], F32)
                    nc.vector.tensor_mul(out=t, in0=psp, in1=rbc)
                    nc.vector.tensor_add(
                        out=t, in0=t, in1=x_sb[:, oc, ic * FB:(ic + 1) * FB])
                    nc.sync.dma_start(out=yr[:, oc, ic * FB:(ic + 1) * FB],
                                      in_=t)

            with tc.tile_pool(name="ps_mm", bufs=2, space="PSUM") as ps_mm:
                att_begin(0)
                for blk in range(NBLK):
                    fcs = range(blk * (N // NBLK // FB),
                                (blk + 1) * (N // NBLK // FB))
                    for oc in range(CCH):
                        for icq in fcs:
                            if icq >= N_IC:
                                continue
                            ps = ps_mm.tile([P, FB], F32)
                            for cc in range(CCH):
                                nc.tensor.matmul(
                                    ps, R(wq_sb[:, cc, oc * P:(oc + 1) * P]),
                                    R(xn_sb[:, cc, icq * FB:(icq + 1) * FB]),
                                    start=(cc == 0), stop=(cc == CCH - 1),
                                )
                            nc.vector.tensor_scalar_add(
                                out=q_sb[:, oc, icq * FB:(icq + 1) * FB], in0=ps,
                                scalar1=qb_sb[:, oc:oc + 1],
                            )
                        for kc in fcs:
                            ps = ps_mm.tile([P, FB], F32)
                            for cc in range(CCH):
                                nc.tensor.matmul(
                                    ps, R(wk_sb[:, cc, oc * P:(oc + 1) * P]),
                                    R(xn_sb[:, cc, kc * FB:(kc + 1) * FB]),
                                    start=(cc == 0), stop=(cc == CCH - 1),
                                )
                            nc.vector.tensor_scalar_add(
                                out=k_sb[:, oc, kc * FB:(kc + 1) * FB], in0=ps,
                                scalar1=kb_sb[:, oc:oc + 1],
                            )
                    for jc in range(blk * (N_JC // NBLK),
                                    (blk + 1) * (N_JC // NBLK)):
                        ps = ps_mm.tile([P, C], F32)
                        for cc in range(CCH):
                            nc.tensor.matmul(
                                ps, R(xn_sb[:, cc, jc * P:(jc + 1) * P]),
                                R(wv_sb[:, cc, :]),
                                start=(cc == 0), stop=(cc == CCH - 1),
                            )
                        nc.vector.tensor_add(out=vT_sb[:, jc, :], in0=ps,
                                             in1=vb_bc)
                    # interleave query-chunk 0 attention for this block's keys
                    att_jcs(range(blk * (N_JC // NBLK),
                                  (blk + 1) * (N_JC // NBLK)))

            with tc.tile_pool(name="ps_p", bufs=2, space="PSUM") as ps_p:
                att_end(ps_p)
                for ic in range(1, N_IC):
                    att_begin(ic)
                    att_jcs(range(N_JC))
                    att_end(ps_p)



def _host_inputs(x, norm_w, norm_b, qkv_w, qkv_b, proj_w, proj_b):
    f = np.float32
    wq = np.ascontiguousarray(qkv_w[0:C].T, dtype=f)
    wk = np.ascontiguousarray(qkv_w[C:2 * C].T, dtype=f)
    wv = np.ascontiguousarray(qkv_w[2 * C:3 * C].T, dtype=f)
    wp = np.ascontiguousarray(proj_w.T, dtype=f)
    qb, kb, vb = (np.ascontiguousarray(qkv_b[i * C:(i + 1) * C], dtype=f)
                  for i in range(3))
    GG = G // CCH
    mask = np.zeros((P, GG), dtype=f)
    mask[np.arange(P), np.arange(P) // (C // G)] = 1.0 / GSIZE
    maskT = np.ascontiguousarray(np.sign(mask.T))

    shared = dict(
        wq=wq, wk=wk, wv=wv, wp=wp, qb=qb, kb=kb, vb=vb,
        pb=np.ascontiguousarray(proj_b, dtype=f),
        nw=np.ascontiguousarray(norm_w, dtype=f),
        nb=np.ascontiguousarray(norm_b, dtype=f),
        mask=mask, maskT=maskT,
    )

    in_maps = []
    for core in range(N_CORES):
        b, h = core // 2, core % 2
        xv = np.asarray(x[b], dtype=f).reshape(C, N)
        xrot = np.ascontiguousarray(np.roll(xv, -h * NQ, axis=1))
        in_maps.append(dict(shared, xb=xrot))
    return in_maps


def kernel(x, norm_w, norm_b, qkv_w, qkv_b, proj_w, proj_b, num_heads=1):
    x, norm_w, norm_b, qkv_w, qkv_b, proj_w, proj_b = (
        np.asarray(a) for a in (x, norm_w, norm_b, qkv_w, qkv_b, proj_w, proj_b))
    nc = _build()
    in_maps = _host_inputs(x, norm_w, norm_b, qkv_w, qkv_b, proj_w, proj_b)
    res = bass_utils.run_bass_kernel_spmd(nc, in_maps, core_ids=list(range(N_CORES)))
    out = np.empty((B, C, N), dtype=np.float32)
    for core in range(N_CORES):
        b, h = core // 2, core % 2
        out[b, :, h * NQ:(h + 1) * NQ] = res.results[core]["y"]
    return out.reshape(B, C, H, W)
